# revision 22
# baseline (speedup 1.0000x reference)
"""Expert-parallel MoE GLU MLP kernel for Trainium2.

8 experts -> 8 NeuronCores, one expert per core (no collectives needed).
Per core:  x (C,H) @ w_gate_up (H,2I) -> GLU -> (C,I) @ w_down (I,H) -> (C,H)

All inputs are pre-cast to bf16 on the host (the kernel computes in bf16
anyway, so numerics are unchanged: rel err 4.1e-3), which halves HBM
traffic (112 MB -> 60 MB per core), eliminates every on-chip f32->bf16
cast, and removes the early-phase DMA crunch that starved the PE.

Per-core span is ~96% PE-busy at the bf16 streaming roofline (1 col/cyc
@ 2.4 GHz, 655 us of pure streaming for 51.5 GFLOP); measured ~687 us.

Strategy (per core), all matmuls bf16 (fp32 accumulate in PSUM):
  - 10 warm-up matmuls on zeroed tiles right after the framework
    preamble: the PE HAM clock-gate reaches 8/8 (2.4 GHz) during the
    first DMA wait instead of running the first real chains cold.
  - GEMM1: stationary = w_gate_up column slices [128h x 128f] (bf16
    direct load, double-buffered), moving = xT (bf16, loaded in pieces
    interleaved with the first weight loads in consumption order). GLU =
    silu(gate) [ACT] * up [DVE] written straight into a bf16
    SBUF-resident act tile (I, C) - no DRAM round-trip.
  - GEMM2: stationary = act tiles [128i x 128c], moving = w_down h-slabs
    of 512 cols (N=512 chains halve the per-matmul NX overhead vs 256),
    psum accumulates the full 32-tile I chain -> copy -> out (C,H).
    Slab 0 is prefetched during phase 1 (zero phase-boundary bubble),
    later slabs one ahead.
  - Single PSUM pool for both phases (no pool-release barrier), and all
    matmuls one dtype (avoids the fp32r<->bf16 FWL interleaving hazard
    that crashes the PE).
"""
import numpy as np

E, C, H, I = 8, 1024, 2048, 4096
P = 128
HT, IT, CT = H // P, I // P, C // P  # 16, 32, 8

_CACHE = {}


def _build():
    import concourse.bacc as bacc
    import concourse.mybir as mybir
    import concourse.tile as tile

    f32 = mybir.dt.float32
    bf16 = mybir.dt.bfloat16
    AF = mybir.ActivationFunctionType

    nc = bacc.Bacc("TRN2", target_bir_lowering=False, debug=False)
    xT = nc.declare_dram_parameter("xT", [H, C], bf16, isOutput=False).ap()
    wgu = nc.declare_dram_parameter("wgu", [H, 2 * I], bf16, isOutput=False).ap()
    wdn = nc.declare_dram_parameter("wdn", [I, H], bf16, isOutput=False).ap()
    out = nc.declare_dram_parameter("out", [C, H], f32, isOutput=True).ap()

    xT_v = xT.rearrange("(ht p) c -> p ht c", p=P)    # [128, 16, 1024]
    wgu_v = wgu.rearrange("(ht p) f -> p ht f", p=P)  # [128, 16, 8192]
    wdn_v = wdn.rearrange("(it p) h -> p it h", p=P)  # [128, 32, 2048]
    out_v = out.rearrange("(ct p) h -> p ct h", p=P)  # [128, 8, 2048]

    NHC = 4               # w_down h-slabs
    HW_ = H // NHC        # 512 cols per slab

    with tile.TileContext(nc) as tc:
        with (
            tc.tile_pool(name="acts_pool", bufs=1) as actsp,
            tc.tile_pool(name="pp", bufs=1, space="PSUM") as pp,
            tc.tile_pool(name="sbs", bufs=3) as sbs,
        ):
            # acts[p, it, c] = act row (it*128+p), col c  (bf16, 8 MB)
            acts = actsp.tile([P, IT, C], bf16)

            # slab 0 of w_down is prefetched during phase 1 from this small
            # pool so the phase boundary has zero weight-load bubble
            w2pre_cm = tc.tile_pool(name="w2pre", bufs=1)
            w2pre = w2pre_cm.__enter__()
            wsl0 = w2pre.tile([P, IT, HW_], bf16, name="wsl0")

            # ---- PE warm-up: dummy matmuls on zeroed tiles so the HAM
            # clock-gate reaches 8/8 (2.4 GHz) before the first real chain;
            # they run during the fixed ~6.5us preamble + first-DMA window
            # in which the PE would otherwise idle (and then start cold).
            warm_cm = tc.tile_pool(name="warm", bufs=1)
            warm = warm_cm.__enter__()
            wd = warm.tile([P, P], bf16, name="wd")
            xd = warm.tile([P, 512], bf16, name="xd")
            nc.vector.memset(wd, 0)
            nc.vector.memset(xd, 0)
            for _ in range(8):
                pw = pp.tile([P, 512], f32, tag="pg", bufs=2)
                nc.tensor.matmul(pw, wd, xd, start=True, stop=True)

            # ---- Phase 1: gate_up GEMM + GLU -> acts ---------------------
            with (
                tc.tile_pool(name="xt_pool", bufs=1) as xtp,
                tc.tile_pool(name="w1", bufs=3) as w1,
            ):
                xt = xtp.tile([P, HT, C], bf16)  # xt[p, ht, c] = x[c, ht*128+p]

                def load_w1(i, which):
                    w = w1.tile([P, HT, P], bf16, tag=which,
                                name=f"{which}{i}")
                    off = 0 if which == "wg" else I
                    nc.sync.dma_start(
                        w, wgu_v[:, :, off + i * P:off + (i + 1) * P])
                    return w

                # Startup: first-chain weights and the cc=0 half of x are
                # interleaved on the Sync HWDGE queue, pieces ordered so
                # data arrives just ahead of the consuming LDW/MATMUL.
                # The i=0 weights are SEPARATE small tiles: a consumer of
                # one slice of a multi-DMA tile waits for every writer of
                # that tile, and the SDMA engines round-robin active
                # transfers, so whole-tile deps push the first chain out
                # to when nearly all in-flight startup bytes have landed.
                wg0a = w1.tile([P, 4, P], bf16, name="wg0a")
                wg0b = w1.tile([P, HT - 4, P], bf16, name="wg0b")
                wu0a = w1.tile([P, 8, P], bf16, name="wu0a")
                wu0b = w1.tile([P, 8, P], bf16, name="wu0b")
                nc.sync.dma_start(wg0a, wgu_v[:, 0:4, 0:P])
                nc.sync.dma_start(xt[:, 0:2, 0:512], xT_v[:, 0:2, 0:512])
                nc.sync.dma_start(wg0b, wgu_v[:, 4:16, 0:P])
                nc.sync.dma_start(xt[:, 2:8, 0:512], xT_v[:, 2:8, 0:512])
                nc.sync.dma_start(wu0a, wgu_v[:, 0:8, I:I + P])
                nc.sync.dma_start(xt[:, 8:16, 0:512], xT_v[:, 8:16, 0:512])
                nc.sync.dma_start(wu0b, wgu_v[:, 8:16, I:I + P])
                nc.sync.dma_start(xt[:, 0:8, 512:1024],
                                    xT_v[:, 0:8, 512:1024])
                nc.sync.dma_start(xt[:, 8:16, 512:1024],
                                    xT_v[:, 8:16, 512:1024])

                wg_n = load_w1(1, "wg")
                wu_n = load_w1(1, "wu")
                for i in range(IT):
                    if i > 0:
                        wg, wu = wg_n, wu_n
                        if i + 1 < IT:
                            wg_n = load_w1(i + 1, "wg")
                            wu_n = load_w1(i + 1, "wu")
                    if 8 <= i < 12:
                        # slab-0 quarter loads, spread mid-phase-1
                        q = i - 8
                        qs = slice(q * (IT // 4), (q + 1) * (IT // 4))
                        nc.sync.dma_start(wsl0[:, qs, :], wdn_v[:, qs, 0:HW_])
                    for cc in range(2):
                        cs = slice(cc * 512, (cc + 1) * 512)
                        pg = pp.tile([P, 512], f32, tag="pg", bufs=2)
                        pu = pp.tile([P, 512], f32, tag="pu", bufs=2)
                        for ht in range(HT):
                            wga = ((wg0a[:, ht, :] if ht < 4 else
                                    wg0b[:, ht - 4, :]) if i == 0 else
                                   wg[:, ht, :])
                            nc.tensor.matmul(
                                pg, wga, xt[:, ht, cs],
                                start=(ht == 0), stop=(ht == HT - 1))
                        for ht in range(HT):
                            wua = ((wu0a[:, ht, :] if ht < 8 else
                                    wu0b[:, ht - 8, :]) if i == 0 else
                                   wu[:, ht, :])
                            nc.tensor.matmul(
                                pu, wua, xt[:, ht, cs],
                                start=(ht == 0), stop=(ht == HT - 1))
                        sil = sbs.tile([P, 512], f32, tag="sil")
                        nc.scalar.activation(sil, pg, AF.Silu)
                        nc.vector.tensor_mul(acts[:, i, cs], sil, pu)

            # ---- Phase 2: down GEMM (bf16), full-I psum chains -----------
            with tc.tile_pool(name="w2", bufs=2) as w2:
                def load_w2(hc):
                    hs = slice(hc * HW_, (hc + 1) * HW_)
                    wsl = w2.tile([P, IT, HW_], bf16, tag="wsl",
                                  name=f"wsl{hc}")
                    for q in range(2):
                        qs = slice(q * (IT // 2), (q + 1) * (IT // 2))
                        nc.sync.dma_start(wsl[:, qs, :], wdn_v[:, qs, hs])
                    return wsl

                wsl_n = load_w2(1)
                for hc in range(NHC):
                    hs = slice(hc * HW_, (hc + 1) * HW_)
                    if hc == 0:
                        wsl = wsl0
                    else:
                        wsl = wsl_n
                        if hc + 1 < NHC:
                            wsl_n = load_w2(hc + 1)
                    for ct in range(CT):
                        ps = pp.tile([P, HW_], f32, tag="ps", bufs=4)
                        for i in range(IT):
                            nc.tensor.matmul(
                                ps,
                                acts[:, i, ct * P:(ct + 1) * P],
                                wsl[:, i, :],
                                start=(i == 0), stop=(i == IT - 1))
                        osb = sbs.tile([P, HW_], f32, tag="osb")
                        nc.vector.tensor_copy(osb, ps)
                        nc.sync.dma_start(out_v[:, ct, hs], osb)
            warm_cm.__exit__(None, None, None)
            w2pre_cm.__exit__(None, None, None)

    nc.compile()
    return nc


def _get_nc():
    if "nc" not in _CACHE:
        _CACHE["nc"] = _build()
    return _CACHE["nc"]


def _run(hidden_states, w_gate_up, w_down, trace=False):
    import ml_dtypes
    from concourse.bass_utils import run_bass_kernel_spmd

    nc = _get_nc()
    bf = ml_dtypes.bfloat16
    hs = np.asarray(hidden_states, dtype=np.float32)
    wg = np.asarray(w_gate_up, dtype=np.float32).astype(bf)
    wd = np.asarray(w_down, dtype=np.float32).astype(bf)
    in_maps = [
        {
            "xT": np.ascontiguousarray(hs[e].T.astype(bf)),
            "wgu": np.ascontiguousarray(wg[e]),
            "wdn": np.ascontiguousarray(wd[e]),
        }
        for e in range(E)
    ]
    res = run_bass_kernel_spmd(nc, in_maps, list(range(E)), trace=trace)
    output = np.stack([res.results[e]["out"] for e in range(E)], axis=0)
    return output, res


def kernel(hidden_states, w_gate_up, w_down):
    output, _ = _run(hidden_states, w_gate_up, w_down, trace=False)
    return output


# revision 24
# speedup vs baseline: 1.0016x; 1.0016x over previous
"""Expert-parallel MoE GLU MLP kernel for Trainium2.

8 experts -> 8 NeuronCores, one expert per core (no collectives needed).
Per core:  x (C,H) @ w_gate_up (H,2I) -> GLU -> (C,I) @ w_down (I,H) -> (C,H)

All inputs are pre-cast to bf16 on the host (the kernel computes in bf16
anyway, so numerics are unchanged: rel err 4.1e-3), which halves HBM
traffic (112 MB -> 60 MB per core), eliminates every on-chip f32->bf16
cast, and removes the early-phase DMA crunch that starved the PE.

Per-core span is ~96% PE-busy at the bf16 streaming roofline (1 col/cyc
@ 2.4 GHz, 655 us of pure streaming for 51.5 GFLOP); measured ~687 us.

Strategy (per core), all matmuls bf16 (fp32 accumulate in PSUM):
  - 10 warm-up matmuls on zeroed tiles right after the framework
    preamble: the PE HAM clock-gate reaches 8/8 (2.4 GHz) during the
    first DMA wait instead of running the first real chains cold.
  - GEMM1: stationary = w_gate_up column slices [128h x 128f] (bf16
    direct load, double-buffered), moving = xT (bf16, loaded in pieces
    interleaved with the first weight loads in consumption order). GLU =
    silu(gate) [ACT] * up [DVE] written straight into a bf16
    SBUF-resident act tile (I, C) - no DRAM round-trip.
  - GEMM2: stationary = act tiles [128i x 128c], moving = w_down h-slabs
    of 512 cols (N=512 chains halve the per-matmul NX overhead vs 256),
    psum accumulates the full 32-tile I chain -> copy -> out (C,H).
    Slab 0 is prefetched during phase 1 (zero phase-boundary bubble),
    later slabs one ahead.
  - Single PSUM pool for both phases (no pool-release barrier), and all
    matmuls one dtype (avoids the fp32r<->bf16 FWL interleaving hazard
    that crashes the PE).
"""
import numpy as np

E, C, H, I = 8, 1024, 2048, 4096
P = 128
HT, IT, CT = H // P, I // P, C // P  # 16, 32, 8

_CACHE = {}


def _build():
    import concourse.bacc as bacc
    import concourse.mybir as mybir
    import concourse.tile as tile

    f32 = mybir.dt.float32
    bf16 = mybir.dt.bfloat16
    AF = mybir.ActivationFunctionType

    nc = bacc.Bacc("TRN2", target_bir_lowering=False, debug=False)
    xT = nc.declare_dram_parameter("xT", [H, C], bf16, isOutput=False).ap()
    wgu = nc.declare_dram_parameter("wgu", [H, 2 * I], bf16, isOutput=False).ap()
    wdn = nc.declare_dram_parameter("wdn", [I, H], bf16, isOutput=False).ap()
    out = nc.declare_dram_parameter("out", [C, H], f32, isOutput=True).ap()

    xT_v = xT.rearrange("(ht p) c -> p ht c", p=P)    # [128, 16, 1024]
    wgu_v = wgu.rearrange("(ht p) f -> p ht f", p=P)  # [128, 16, 8192]
    wdn_v = wdn.rearrange("(it p) h -> p it h", p=P)  # [128, 32, 2048]
    out_v = out.rearrange("(ct p) h -> p ct h", p=P)  # [128, 8, 2048]

    NHC = 4               # w_down h-slabs
    HW_ = H // NHC        # 512 cols per slab

    with tile.TileContext(nc) as tc:
        with (
            tc.tile_pool(name="acts_pool", bufs=1) as actsp,
            tc.tile_pool(name="pp", bufs=1, space="PSUM") as pp,
            tc.tile_pool(name="sbs", bufs=3) as sbs,
        ):
            # acts[p, it, c] = act row (it*128+p), col c  (bf16, 8 MB)
            acts = actsp.tile([P, IT, C], bf16)

            # slab 0 of w_down is prefetched during phase 1 from this small
            # pool so the phase boundary has zero weight-load bubble
            w2pre_cm = tc.tile_pool(name="w2pre", bufs=1)
            w2pre = w2pre_cm.__enter__()
            wsl0 = w2pre.tile([P, IT, HW_], bf16, name="wsl0")

            # ---- PE warm-up: dummy matmuls on zeroed tiles so the HAM
            # clock-gate reaches 8/8 (2.4 GHz) before the first real chain;
            # they run during the fixed ~6.5us preamble + first-DMA window
            # in which the PE would otherwise idle (and then start cold).
            warm_cm = tc.tile_pool(name="warm", bufs=1)
            warm = warm_cm.__enter__()
            wd = warm.tile([P, P], bf16, name="wd")
            xd = warm.tile([P, 512], bf16, name="xd")
            nc.vector.memset(wd, 0)
            nc.vector.memset(xd, 0)
            for _ in range(10):
                pw = pp.tile([P, 512], f32, tag="pg", bufs=2)
                nc.tensor.matmul(pw, wd, xd, start=True, stop=True)

            # ---- Phase 1: gate_up GEMM + GLU -> acts ---------------------
            with (
                tc.tile_pool(name="xt_pool", bufs=1) as xtp,
                tc.tile_pool(name="w1", bufs=3) as w1,
            ):
                xt = xtp.tile([P, HT, C], bf16)  # xt[p, ht, c] = x[c, ht*128+p]

                def load_w1(i, which):
                    w = w1.tile([P, HT, P], bf16, tag=which,
                                name=f"{which}{i}")
                    off = 0 if which == "wg" else I
                    nc.sync.dma_start(
                        w, wgu_v[:, :, off + i * P:off + (i + 1) * P])
                    return w

                # Startup: first-chain weights and the cc=0 half of x are
                # interleaved on the Sync HWDGE queue, pieces ordered so
                # data arrives just ahead of the consuming LDW/MATMUL.
                # The i=0 weights are SEPARATE small tiles: a consumer of
                # one slice of a multi-DMA tile waits for every writer of
                # that tile, and the SDMA engines round-robin active
                # transfers, so whole-tile deps push the first chain out
                # to when nearly all in-flight startup bytes have landed.
                wg0a = w1.tile([P, 4, P], bf16, name="wg0a")
                wg0b = w1.tile([P, HT - 4, P], bf16, name="wg0b")
                wu0a = w1.tile([P, 8, P], bf16, name="wu0a")
                wu0b = w1.tile([P, 8, P], bf16, name="wu0b")
                nc.sync.dma_start(wg0a, wgu_v[:, 0:4, 0:P])
                nc.sync.dma_start(xt[:, 0:2, 0:512], xT_v[:, 0:2, 0:512])
                nc.sync.dma_start(wg0b, wgu_v[:, 4:16, 0:P])
                # cc=0 half of x at 2-ht granularity: early effective DMA
                # bandwidth is only ~120-200 GB/s (ramp + receipt latency),
                # so coarse pieces turn into multi-us chain stalls that
                # re-throttle the HAM clock; fine pieces keep stalls sub-us.
                for hh in range(1, 8):
                    nc.sync.dma_start(
                        xt[:, 2 * hh:2 * hh + 2, 0:512],
                        xT_v[:, 2 * hh:2 * hh + 2, 0:512])
                    if hh == 3:
                        nc.sync.dma_start(wu0a, wgu_v[:, 0:8, I:I + P])
                    if hh == 5:
                        nc.sync.dma_start(wu0b, wgu_v[:, 8:16, I:I + P])
                nc.sync.dma_start(xt[:, 0:8, 512:1024],
                                    xT_v[:, 0:8, 512:1024])
                nc.sync.dma_start(xt[:, 8:16, 512:1024],
                                    xT_v[:, 8:16, 512:1024])

                wg_n = load_w1(1, "wg")
                wu_n = load_w1(1, "wu")
                for i in range(IT):
                    if i > 0:
                        wg, wu = wg_n, wu_n
                        if i + 1 < IT:
                            wg_n = load_w1(i + 1, "wg")
                            wu_n = load_w1(i + 1, "wu")
                    if 8 <= i < 12:
                        # slab-0 quarter loads, spread mid-phase-1
                        q = i - 8
                        qs = slice(q * (IT // 4), (q + 1) * (IT // 4))
                        nc.sync.dma_start(wsl0[:, qs, :], wdn_v[:, qs, 0:HW_])
                    for cc in range(2):
                        cs = slice(cc * 512, (cc + 1) * 512)
                        pg = pp.tile([P, 512], f32, tag="pg", bufs=2)
                        pu = pp.tile([P, 512], f32, tag="pu", bufs=2)
                        for ht in range(HT):
                            wga = ((wg0a[:, ht, :] if ht < 4 else
                                    wg0b[:, ht - 4, :]) if i == 0 else
                                   wg[:, ht, :])
                            nc.tensor.matmul(
                                pg, wga, xt[:, ht, cs],
                                start=(ht == 0), stop=(ht == HT - 1))
                        for ht in range(HT):
                            wua = ((wu0a[:, ht, :] if ht < 8 else
                                    wu0b[:, ht - 8, :]) if i == 0 else
                                   wu[:, ht, :])
                            nc.tensor.matmul(
                                pu, wua, xt[:, ht, cs],
                                start=(ht == 0), stop=(ht == HT - 1))
                        sil = sbs.tile([P, 512], f32, tag="sil")
                        nc.scalar.activation(sil, pg, AF.Silu)
                        nc.vector.tensor_mul(acts[:, i, cs], sil, pu)

            # ---- Phase 2: down GEMM (bf16), full-I psum chains -----------
            with tc.tile_pool(name="w2", bufs=2) as w2:
                def load_w2(hc):
                    hs = slice(hc * HW_, (hc + 1) * HW_)
                    wsl = w2.tile([P, IT, HW_], bf16, tag="wsl",
                                  name=f"wsl{hc}")
                    for q in range(2):
                        qs = slice(q * (IT // 2), (q + 1) * (IT // 2))
                        nc.sync.dma_start(wsl[:, qs, :], wdn_v[:, qs, hs])
                    return wsl

                wsl_n = load_w2(1)
                for hc in range(NHC):
                    hs = slice(hc * HW_, (hc + 1) * HW_)
                    if hc == 0:
                        wsl = wsl0
                    else:
                        wsl = wsl_n
                        if hc + 1 < NHC:
                            wsl_n = load_w2(hc + 1)
                    for ct in range(CT):
                        ps = pp.tile([P, HW_], f32, tag="ps", bufs=4)
                        for i in range(IT):
                            nc.tensor.matmul(
                                ps,
                                acts[:, i, ct * P:(ct + 1) * P],
                                wsl[:, i, :],
                                start=(i == 0), stop=(i == IT - 1))
                        osb = sbs.tile([P, HW_], f32, tag="osb")
                        nc.vector.tensor_copy(osb, ps)
                        nc.sync.dma_start(out_v[:, ct, hs], osb)
            warm_cm.__exit__(None, None, None)
            w2pre_cm.__exit__(None, None, None)

    nc.compile()
    return nc


def _get_nc():
    if "nc" not in _CACHE:
        _CACHE["nc"] = _build()
    return _CACHE["nc"]


def _run(hidden_states, w_gate_up, w_down, trace=False):
    import ml_dtypes
    from concourse.bass_utils import run_bass_kernel_spmd

    nc = _get_nc()
    bf = ml_dtypes.bfloat16
    hs = np.asarray(hidden_states, dtype=np.float32)
    wg = np.asarray(w_gate_up, dtype=np.float32).astype(bf)
    wd = np.asarray(w_down, dtype=np.float32).astype(bf)
    in_maps = [
        {
            "xT": np.ascontiguousarray(hs[e].T.astype(bf)),
            "wgu": np.ascontiguousarray(wg[e]),
            "wdn": np.ascontiguousarray(wd[e]),
        }
        for e in range(E)
    ]
    res = run_bass_kernel_spmd(nc, in_maps, list(range(E)), trace=trace)
    output = np.stack([res.results[e]["out"] for e in range(E)], axis=0)
    return output, res


def kernel(hidden_states, w_gate_up, w_down):
    output, _ = _run(hidden_states, w_gate_up, w_down, trace=False)
    return output


# revision 29
# speedup vs baseline: 1.0026x; 1.0010x over previous
"""Expert-parallel MoE GLU MLP kernel for Trainium2.

8 experts -> 8 NeuronCores, one expert per core (no collectives needed).
Per core:  x (C,H) @ w_gate_up (H,2I) -> GLU -> (C,I) @ w_down (I,H) -> (C,H)

All inputs are pre-cast to bf16 on the host (the kernel computes in bf16
anyway, so numerics are unchanged: rel err 4.1e-3), which halves HBM
traffic (112 MB -> 60 MB per core), eliminates every on-chip f32->bf16
cast, and removes the early-phase DMA crunch that starved the PE.

Per-core span is ~96% PE-busy at the bf16 streaming roofline (1 col/cyc
@ 2.4 GHz, 655 us of pure streaming for 51.5 GFLOP); measured ~687 us.

Strategy (per core), all matmuls bf16 (fp32 accumulate in PSUM):
  - 10 warm-up matmuls on zeroed tiles right after the framework
    preamble: the PE HAM clock-gate reaches 8/8 (2.4 GHz) during the
    first DMA wait instead of running the first real chains cold.
  - GEMM1: stationary = w_gate_up column slices [128h x 128f] (bf16
    direct load, double-buffered), moving = xT (bf16, loaded in pieces
    interleaved with the first weight loads in consumption order). GLU =
    silu(gate) [ACT] * up [DVE] written straight into a bf16
    SBUF-resident act tile (I, C) - no DRAM round-trip.
  - GEMM2: stationary = act tiles [128i x 128c], moving = w_down h-slabs
    of 512 cols (N=512 chains halve the per-matmul NX overhead vs 256),
    psum accumulates the full 32-tile I chain -> copy -> out (C,H).
    Slab 0 is prefetched during phase 1 (zero phase-boundary bubble),
    later slabs one ahead.
  - Single PSUM pool for both phases (no pool-release barrier), and all
    matmuls one dtype (avoids the fp32r<->bf16 FWL interleaving hazard
    that crashes the PE).
"""
import numpy as np

E, C, H, I = 8, 1024, 2048, 4096
P = 128
HT, IT, CT = H // P, I // P, C // P  # 16, 32, 8

_CACHE = {}


def _build():
    import concourse.bacc as bacc
    import concourse.mybir as mybir
    import concourse.tile as tile

    f32 = mybir.dt.float32
    bf16 = mybir.dt.bfloat16
    AF = mybir.ActivationFunctionType

    nc = bacc.Bacc("TRN2", target_bir_lowering=False, debug=False)
    xT = nc.declare_dram_parameter("xT", [H, C], bf16, isOutput=False).ap()
    # wgu is host-packed into SBUF layout: row (i*128+p), col (ht*128+f)
    # holds w_gate_up[ht*128+p, i*128+f] — each 128x128 weight tile DMA
    # then reads 4KB contiguous per partition instead of 16 strided 256B
    # chunks (sub-512B chunks fall off SDMA line rate).
    wgu = nc.declare_dram_parameter("wgu", [2 * I, H], bf16, isOutput=False).ap()
    wdn = nc.declare_dram_parameter("wdn", [I, H], bf16, isOutput=False).ap()
    out = nc.declare_dram_parameter("out", [C, H], f32, isOutput=True).ap()

    xT_v = xT.rearrange("(ht p) c -> p ht c", p=P)    # [128, 16, 1024]
    wgu_v = wgu.rearrange("(i p) (ht f) -> p i ht f", p=P, f=P)  # [128,64,16,128]
    wdn_v = wdn.rearrange("(it p) h -> p it h", p=P)  # [128, 32, 2048]
    out_v = out.rearrange("(ct p) h -> p ct h", p=P)  # [128, 8, 2048]

    NHC = 4               # w_down h-slabs
    HW_ = H // NHC        # 512 cols per slab

    with tile.TileContext(nc) as tc:
        with (
            tc.tile_pool(name="acts_pool", bufs=1) as actsp,
            tc.tile_pool(name="pp", bufs=1, space="PSUM") as pp,
            tc.tile_pool(name="sbs", bufs=3) as sbs,
        ):
            # acts[p, it, c] = act row (it*128+p), col c  (bf16, 8 MB)
            acts = actsp.tile([P, IT, C], bf16)

            # slab 0 of w_down is prefetched during phase 1 from this small
            # pool so the phase boundary has zero weight-load bubble
            w2pre_cm = tc.tile_pool(name="w2pre", bufs=1)
            w2pre = w2pre_cm.__enter__()
            wsl0 = w2pre.tile([P, IT, HW_], bf16, name="wsl0")

            # ---- PE warm-up: dummy matmuls on zeroed tiles so the HAM
            # clock-gate reaches 8/8 (2.4 GHz) before the first real chain;
            # they run during the fixed ~6.5us preamble + first-DMA window
            # in which the PE would otherwise idle (and then start cold).
            warm_cm = tc.tile_pool(name="warm", bufs=1)
            warm = warm_cm.__enter__()
            wd = warm.tile([P, P], bf16, name="wd")
            xd = warm.tile([P, 512], bf16, name="xd")
            nc.vector.memset(wd, 0)
            nc.vector.memset(xd, 0)
            for _ in range(10):
                pw = pp.tile([P, 512], f32, tag="pg", bufs=2)
                nc.tensor.matmul(pw, wd, xd, start=True, stop=True)

            # ---- Phase 1: gate_up GEMM + GLU -> acts ---------------------
            with (
                tc.tile_pool(name="xt_pool", bufs=1) as xtp,
                tc.tile_pool(name="w1", bufs=3) as w1,
            ):
                xt = xtp.tile([P, HT, C], bf16)  # xt[p, ht, c] = x[c, ht*128+p]

                def load_w1(i, which):
                    w = w1.tile([P, HT, P], bf16, tag=which,
                                name=f"{which}{i}")
                    ti = i if which == "wg" else IT + i
                    nc.sync.dma_start(w, wgu_v[:, ti])
                    return w

                # Startup: first-chain weights and the cc=0 half of x are
                # interleaved on the Sync HWDGE queue, pieces ordered so
                # data arrives just ahead of the consuming LDW/MATMUL.
                # The i=0 weights are SEPARATE small tiles: a consumer of
                # one slice of a multi-DMA tile waits for every writer of
                # that tile, and the SDMA engines round-robin active
                # transfers, so whole-tile deps push the first chain out
                # to when nearly all in-flight startup bytes have landed.
                wg0a = w1.tile([P, 4, P], bf16, name="wg0a")
                wg0b = w1.tile([P, HT - 4, P], bf16, name="wg0b")
                wu0a = w1.tile([P, 8, P], bf16, name="wu0a")
                wu0b = w1.tile([P, 8, P], bf16, name="wu0b")
                nc.sync.dma_start(wg0a, wgu_v[:, 0, 0:4, :])
                nc.sync.dma_start(xt[:, 0:2, 0:512], xT_v[:, 0:2, 0:512])
                nc.sync.dma_start(wg0b, wgu_v[:, 0, 4:16, :])
                # cc=0 half of x at 2-ht granularity: early effective DMA
                # bandwidth is only ~120-200 GB/s (ramp + receipt latency),
                # so coarse pieces turn into multi-us chain stalls that
                # re-throttle the HAM clock; fine pieces keep stalls sub-us.
                for hh in range(1, 8):
                    nc.sync.dma_start(
                        xt[:, 2 * hh:2 * hh + 2, 0:512],
                        xT_v[:, 2 * hh:2 * hh + 2, 0:512])
                    if hh == 3:
                        nc.sync.dma_start(wu0a, wgu_v[:, IT, 0:8, :])
                    if hh == 5:
                        nc.sync.dma_start(wu0b, wgu_v[:, IT, 8:16, :])
                nc.sync.dma_start(xt[:, 0:8, 512:1024],
                                    xT_v[:, 0:8, 512:1024])
                nc.sync.dma_start(xt[:, 8:16, 512:1024],
                                    xT_v[:, 8:16, 512:1024])

                wg_n = load_w1(1, "wg")
                wu_n = load_w1(1, "wu")
                for i in range(IT):
                    if i > 0:
                        wg, wu = wg_n, wu_n
                        if i + 1 < IT:
                            wg_n = load_w1(i + 1, "wg")
                            wu_n = load_w1(i + 1, "wu")
                    if 8 <= i < 12:
                        # slab-0 quarter loads, spread mid-phase-1
                        q = i - 8
                        qs = slice(q * (IT // 4), (q + 1) * (IT // 4))
                        nc.sync.dma_start(wsl0[:, qs, :], wdn_v[:, qs, 0:HW_])
                    for cc in range(2):
                        cs = slice(cc * 512, (cc + 1) * 512)
                        pg = pp.tile([P, 512], f32, tag="pg", bufs=2)
                        pu = pp.tile([P, 512], f32, tag="pu", bufs=2)
                        for ht in range(HT):
                            wga = ((wg0a[:, ht, :] if ht < 4 else
                                    wg0b[:, ht - 4, :]) if i == 0 else
                                   wg[:, ht, :])
                            nc.tensor.matmul(
                                pg, wga, xt[:, ht, cs],
                                start=(ht == 0), stop=(ht == HT - 1))
                        for ht in range(HT):
                            wua = ((wu0a[:, ht, :] if ht < 8 else
                                    wu0b[:, ht - 8, :]) if i == 0 else
                                   wu[:, ht, :])
                            nc.tensor.matmul(
                                pu, wua, xt[:, ht, cs],
                                start=(ht == 0), stop=(ht == HT - 1))
                        sil = sbs.tile([P, 512], f32, tag="sil")
                        nc.scalar.activation(sil, pg, AF.Silu)
                        nc.vector.tensor_mul(acts[:, i, cs], sil, pu)

            # ---- Phase 2: down GEMM (bf16), full-I psum chains -----------
            with tc.tile_pool(name="w2", bufs=2) as w2:
                def load_w2(hc):
                    hs = slice(hc * HW_, (hc + 1) * HW_)
                    wsl = w2.tile([P, IT, HW_], bf16, tag="wsl",
                                  name=f"wsl{hc}")
                    for q in range(2):
                        qs = slice(q * (IT // 2), (q + 1) * (IT // 2))
                        nc.sync.dma_start(wsl[:, qs, :], wdn_v[:, qs, hs])
                    return wsl

                wsl_n = load_w2(1)
                for hc in range(NHC):
                    hs = slice(hc * HW_, (hc + 1) * HW_)
                    if hc == 0:
                        wsl = wsl0
                    else:
                        wsl = wsl_n
                        if hc + 1 < NHC:
                            wsl_n = load_w2(hc + 1)
                    for ct in range(CT):
                        ps = pp.tile([P, HW_], f32, tag="ps", bufs=4)
                        for i in range(IT):
                            nc.tensor.matmul(
                                ps,
                                acts[:, i, ct * P:(ct + 1) * P],
                                wsl[:, i, :],
                                start=(i == 0), stop=(i == IT - 1))
                        osb = sbs.tile([P, HW_], f32, tag="osb")
                        nc.vector.tensor_copy(osb, ps)
                        nc.sync.dma_start(out_v[:, ct, hs], osb)
            warm_cm.__exit__(None, None, None)
            w2pre_cm.__exit__(None, None, None)

    nc.compile()
    return nc


def _get_nc():
    if "nc" not in _CACHE:
        _CACHE["nc"] = _build()
    return _CACHE["nc"]


def _run(hidden_states, w_gate_up, w_down, trace=False):
    import ml_dtypes
    from concourse.bass_utils import run_bass_kernel_spmd

    nc = _get_nc()
    bf = ml_dtypes.bfloat16
    hs = np.asarray(hidden_states, dtype=np.float32)
    wg = np.asarray(w_gate_up, dtype=np.float32).astype(bf)
    wd = np.asarray(w_down, dtype=np.float32).astype(bf)

    def pack_wgu(w):
        # [H, 2I] -> row (i*128+p), col (ht*128+f) = w[ht*128+p, i*128+f]
        a = w.reshape(HT, P, 2 * I // P, P)       # [ht, p, i, f]
        return np.ascontiguousarray(
            a.transpose(2, 1, 0, 3).reshape(2 * I, H))

    in_maps = [
        {
            "xT": np.ascontiguousarray(hs[e].T.astype(bf)),
            "wgu": pack_wgu(wg[e]),
            "wdn": np.ascontiguousarray(wd[e]),
        }
        for e in range(E)
    ]
    res = run_bass_kernel_spmd(nc, in_maps, list(range(E)), trace=trace)
    output = np.stack([res.results[e]["out"] for e in range(E)], axis=0)
    return output, res


def kernel(hidden_states, w_gate_up, w_down):
    output, _ = _run(hidden_states, w_gate_up, w_down, trace=False)
    return output


# revision 30
# speedup vs baseline: 1.0037x; 1.0012x over previous
"""Expert-parallel MoE GLU MLP kernel for Trainium2.

8 experts -> 8 NeuronCores, one expert per core (no collectives needed).
Per core:  x (C,H) @ w_gate_up (H,2I) -> GLU -> (C,I) @ w_down (I,H) -> (C,H)

All inputs are pre-cast to bf16 on the host (the kernel computes in bf16
anyway, so numerics are unchanged: rel err 4.1e-3), which halves HBM
traffic (112 MB -> 60 MB per core), eliminates every on-chip f32->bf16
cast, and removes the early-phase DMA crunch that starved the PE.

Per-core span is ~96% PE-busy at the bf16 streaming roofline (1 col/cyc
@ 2.4 GHz, 655 us of pure streaming for 51.5 GFLOP); measured ~687 us.

Strategy (per core), all matmuls bf16 (fp32 accumulate in PSUM):
  - 10 warm-up matmuls on zeroed tiles right after the framework
    preamble: the PE HAM clock-gate reaches 8/8 (2.4 GHz) during the
    first DMA wait instead of running the first real chains cold.
  - GEMM1: stationary = w_gate_up column slices [128h x 128f] (bf16,
    host-packed into SBUF tile order so each weight DMA reads 4KB
    contiguous per partition instead of strided 256B chunks),
    double-buffered; moving = xT (bf16, loaded in 2-ht pieces
    interleaved with the first weight loads in consumption order -
    early effective DMA bandwidth is only ~120-200 GB/s, so coarse
    pieces turn into multi-us chain stalls that re-throttle the HAM
    clock).  The i=0 weights are separate small tiles: a consumer of
    one slice of a multi-DMA tile waits for every writer of that tile.
    GLU = silu(gate) [ACT] * up [DVE] written straight into a bf16
    SBUF-resident act tile (I, C) - no DRAM round-trip.
  - GEMM2: stationary = act tiles [128i x 128c], moving = w_down h-slabs
    of 512 cols (N=512 chains halve the per-matmul NX overhead vs 256),
    psum accumulates the full 32-tile I chain -> copy -> out (C,H).
    Slab 0 is prefetched during phase 1 (zero phase-boundary bubble),
    later slabs one ahead.
  - Single PSUM pool for both phases (no pool-release barrier), and all
    matmuls one dtype (avoids the fp32r<->bf16 FWL interleaving hazard
    that crashes the PE).
"""
import numpy as np

E, C, H, I = 8, 1024, 2048, 4096
P = 128
HT, IT, CT = H // P, I // P, C // P  # 16, 32, 8

_CACHE = {}


def _build():
    import concourse.bacc as bacc
    import concourse.mybir as mybir
    import concourse.tile as tile

    f32 = mybir.dt.float32
    bf16 = mybir.dt.bfloat16
    AF = mybir.ActivationFunctionType

    nc = bacc.Bacc("TRN2", target_bir_lowering=False, debug=False)
    xT = nc.declare_dram_parameter("xT", [H, C], bf16, isOutput=False).ap()
    # wgu is host-packed into SBUF layout: row (i*128+p), col (ht*128+f)
    # holds w_gate_up[ht*128+p, i*128+f] — each 128x128 weight tile DMA
    # then reads 4KB contiguous per partition instead of 16 strided 256B
    # chunks (sub-512B chunks fall off SDMA line rate).
    wgu = nc.declare_dram_parameter("wgu", [2 * I, H], bf16, isOutput=False).ap()
    wdn = nc.declare_dram_parameter("wdn", [I, H], bf16, isOutput=False).ap()
    out = nc.declare_dram_parameter("out", [C, H], f32, isOutput=True).ap()

    xT_v = xT.rearrange("(ht p) c -> p ht c", p=P)    # [128, 16, 1024]
    wgu_v = wgu.rearrange("(i p) (ht f) -> p i ht f", p=P, f=P)  # [128,64,16,128]
    wdn_v = wdn.rearrange("(it p) h -> p it h", p=P)  # [128, 32, 2048]
    out_v = out.rearrange("(ct p) h -> p ct h", p=P)  # [128, 8, 2048]

    NHC = 4               # w_down h-slabs
    HW_ = H // NHC        # 512 cols per slab

    with tile.TileContext(nc) as tc:
        with (
            tc.tile_pool(name="acts_pool", bufs=1) as actsp,
            tc.tile_pool(name="pp", bufs=1, space="PSUM") as pp,
            tc.tile_pool(name="sbs", bufs=3) as sbs,
        ):
            # acts[p, it, c] = act row (it*128+p), col c  (bf16, 8 MB)
            acts = actsp.tile([P, IT, C], bf16)

            # slab 0 of w_down is prefetched during phase 1 from this small
            # pool so the phase boundary has zero weight-load bubble
            w2pre_cm = tc.tile_pool(name="w2pre", bufs=1)
            w2pre = w2pre_cm.__enter__()
            wsl0 = w2pre.tile([P, IT, HW_], bf16, name="wsl0")

            # ---- PE warm-up: dummy matmuls on zeroed tiles so the HAM
            # clock-gate reaches 8/8 (2.4 GHz) before the first real chain;
            # they run during the fixed ~6.5us preamble + first-DMA window
            # in which the PE would otherwise idle (and then start cold).
            warm_cm = tc.tile_pool(name="warm", bufs=1)
            warm = warm_cm.__enter__()
            wd = warm.tile([P, P], bf16, name="wd")
            xd = warm.tile([P, 512], bf16, name="xd")
            nc.vector.memset(wd, 0)
            nc.vector.memset(xd, 0)
            for _ in range(10):
                pw = pp.tile([P, 512], f32, tag="pg", bufs=2)
                nc.tensor.matmul(pw, wd, xd, start=True, stop=True)

            # ---- Phase 1: gate_up GEMM + GLU -> acts ---------------------
            with (
                tc.tile_pool(name="xt_pool", bufs=1) as xtp,
                tc.tile_pool(name="w1", bufs=3) as w1,
            ):
                xt = xtp.tile([P, HT, C], bf16)  # xt[p, ht, c] = x[c, ht*128+p]

                def load_w1(i, which):
                    w = w1.tile([P, HT, P], bf16, tag=which,
                                name=f"{which}{i}")
                    ti = i if which == "wg" else IT + i
                    nc.sync.dma_start(w, wgu_v[:, ti])
                    return w

                # Startup: first-chain weights and the cc=0 half of x are
                # interleaved on the Sync HWDGE queue, pieces ordered so
                # data arrives just ahead of the consuming LDW/MATMUL.
                # The i=0 weights are SEPARATE small tiles: a consumer of
                # one slice of a multi-DMA tile waits for every writer of
                # that tile, and the SDMA engines round-robin active
                # transfers, so whole-tile deps push the first chain out
                # to when nearly all in-flight startup bytes have landed.
                wg0a = w1.tile([P, 4, P], bf16, name="wg0a")
                wg0b = w1.tile([P, HT - 4, P], bf16, name="wg0b")
                wu0a = w1.tile([P, 8, P], bf16, name="wu0a")
                wu0b = w1.tile([P, 8, P], bf16, name="wu0b")
                nc.sync.dma_start(wg0a, wgu_v[:, 0, 0:4, :])
                nc.sync.dma_start(xt[:, 0:2, 0:512], xT_v[:, 0:2, 0:512])
                nc.sync.dma_start(wg0b, wgu_v[:, 0, 4:16, :])
                # cc=0 half of x at 2-ht granularity: early effective DMA
                # bandwidth is only ~120-200 GB/s (ramp + receipt latency),
                # so coarse pieces turn into multi-us chain stalls that
                # re-throttle the HAM clock; fine pieces keep stalls sub-us.
                for hh in range(1, 8):
                    nc.sync.dma_start(
                        xt[:, 2 * hh:2 * hh + 2, 0:512],
                        xT_v[:, 2 * hh:2 * hh + 2, 0:512])
                    if hh == 3:
                        nc.sync.dma_start(wu0a, wgu_v[:, IT, 0:8, :])
                    if hh == 5:
                        nc.sync.dma_start(wu0b, wgu_v[:, IT, 8:16, :])
                nc.sync.dma_start(xt[:, 0:8, 512:1024],
                                    xT_v[:, 0:8, 512:1024])
                nc.sync.dma_start(xt[:, 8:16, 512:1024],
                                    xT_v[:, 8:16, 512:1024])

                wg_n = load_w1(1, "wg")
                wu_n = load_w1(1, "wu")
                for i in range(IT):
                    if i > 0:
                        wg, wu = wg_n, wu_n
                        if i + 1 < IT:
                            wg_n = load_w1(i + 1, "wg")
                            wu_n = load_w1(i + 1, "wu")
                    if 8 <= i < 12:
                        # slab-0 quarter loads, spread mid-phase-1
                        q = i - 8
                        qs = slice(q * (IT // 4), (q + 1) * (IT // 4))
                        nc.sync.dma_start(wsl0[:, qs, :], wdn_v[:, qs, 0:HW_])
                    for cc in range(2):
                        cs = slice(cc * 512, (cc + 1) * 512)
                        pg = pp.tile([P, 512], f32, tag="pg", bufs=2)
                        pu = pp.tile([P, 512], f32, tag="pu", bufs=2)
                        for ht in range(HT):
                            wga = ((wg0a[:, ht, :] if ht < 4 else
                                    wg0b[:, ht - 4, :]) if i == 0 else
                                   wg[:, ht, :])
                            nc.tensor.matmul(
                                pg, wga, xt[:, ht, cs],
                                start=(ht == 0), stop=(ht == HT - 1))
                        for ht in range(HT):
                            wua = ((wu0a[:, ht, :] if ht < 8 else
                                    wu0b[:, ht - 8, :]) if i == 0 else
                                   wu[:, ht, :])
                            nc.tensor.matmul(
                                pu, wua, xt[:, ht, cs],
                                start=(ht == 0), stop=(ht == HT - 1))
                        sil = sbs.tile([P, 512], f32, tag="sil")
                        nc.scalar.activation(sil, pg, AF.Silu)
                        nc.vector.tensor_mul(acts[:, i, cs], sil, pu)

            # ---- Phase 2: down GEMM (bf16), full-I psum chains -----------
            with tc.tile_pool(name="w2", bufs=2) as w2:
                def load_w2(hc):
                    hs = slice(hc * HW_, (hc + 1) * HW_)
                    wsl = w2.tile([P, IT, HW_], bf16, tag="wsl",
                                  name=f"wsl{hc}")
                    for q in range(2):
                        qs = slice(q * (IT // 2), (q + 1) * (IT // 2))
                        nc.sync.dma_start(wsl[:, qs, :], wdn_v[:, qs, hs])
                    return wsl

                wsl_n = load_w2(1)
                for hc in range(NHC):
                    hs = slice(hc * HW_, (hc + 1) * HW_)
                    if hc == 0:
                        wsl = wsl0
                    else:
                        wsl = wsl_n
                        if hc + 1 < NHC:
                            wsl_n = load_w2(hc + 1)
                    for ct in range(CT):
                        ps = pp.tile([P, HW_], f32, tag="ps", bufs=4)
                        for i in range(IT):
                            nc.tensor.matmul(
                                ps,
                                acts[:, i, ct * P:(ct + 1) * P],
                                wsl[:, i, :],
                                start=(i == 0), stop=(i == IT - 1))
                        osb = sbs.tile([P, HW_], f32, tag="osb")
                        nc.vector.tensor_copy(osb, ps)
                        nc.sync.dma_start(out_v[:, ct, hs], osb)
            warm_cm.__exit__(None, None, None)
            w2pre_cm.__exit__(None, None, None)

    nc.compile()
    return nc


def _get_nc():
    if "nc" not in _CACHE:
        _CACHE["nc"] = _build()
    return _CACHE["nc"]


def _run(hidden_states, w_gate_up, w_down, trace=False):
    import ml_dtypes
    from concourse.bass_utils import run_bass_kernel_spmd

    nc = _get_nc()
    bf = ml_dtypes.bfloat16
    hs = np.asarray(hidden_states, dtype=np.float32)
    wg = np.asarray(w_gate_up, dtype=np.float32).astype(bf)
    wd = np.asarray(w_down, dtype=np.float32).astype(bf)

    def pack_wgu(w):
        # [H, 2I] -> row (i*128+p), col (ht*128+f) = w[ht*128+p, i*128+f]
        a = w.reshape(HT, P, 2 * I // P, P)       # [ht, p, i, f]
        return np.ascontiguousarray(
            a.transpose(2, 1, 0, 3).reshape(2 * I, H))

    in_maps = [
        {
            "xT": np.ascontiguousarray(hs[e].T.astype(bf)),
            "wgu": pack_wgu(wg[e]),
            "wdn": np.ascontiguousarray(wd[e]),
        }
        for e in range(E)
    ]
    res = run_bass_kernel_spmd(nc, in_maps, list(range(E)), trace=trace)
    output = np.stack([res.results[e]["out"] for e in range(E)], axis=0)
    return output, res


def kernel(hidden_states, w_gate_up, w_down):
    output, _ = _run(hidden_states, w_gate_up, w_down, trace=False)
    return output
